# revision 9
# baseline (speedup 1.0000x reference)
"""LocalBandSimilarityBlock — Trainium2 Bass kernel, 8-way sequence-parallel.

Strategy: sort nodes by grid-x (host-side sharding permutation). After the
sort, every node's radius-2 neighbourhood lies within a +-256-row band in
sorted order (verified per-input on host). Each of the 8 cores owns 768
sorted query rows plus a 256-row halo on each side (1280 key rows total,
zero-padded at the edges) — so cores run fully independently, no
collectives. Per 128-query block, attention is computed against a 640-row
key window instead of all 6144 keys (12x less tensor work than dense).

On-device math (per core): LN1 -> q/k/v projections (bf16 matmuls, fp32
accum) -> band-masked augmented-logit attention (q.k/sqrt(D) + cos-sim via
concatenated [q | hn] contraction) -> fp32 softmax with isolated-node
fallback to v -> residual + Wo -> LN2 -> FFN(gelu) -> residual. The grid
band mask (Chebyshev distance <= 2, self excluded) is computed on device
from the grid coordinates.
"""
import os
import sys

import numpy as np

sys.path.insert(0, "/opt/trn_rl_repo")

import ml_dtypes  # noqa: E402
from contextlib import ExitStack  # noqa: E402

import concourse.bacc as bacc  # noqa: E402
import concourse.bass as bass  # noqa: E402
import concourse.tile as tile  # noqa: E402
from concourse import mybir  # noqa: E402
from concourse.bass_utils import run_bass_kernel_spmd  # noqa: E402

P = 128
D = 512
DC = D // P            # 4 d-chunks
DFF = 2048
FC = DFF // P          # 16 ffn chunks
N = 6144
N_CORES = 8
ROWS = N // N_CORES    # 768 query rows per core
NB = ROWS // P         # 6 query blocks per core
H = 256                # halo rows each side (multiple of 128)
KR = ROWS + 2 * H      # 1280 key rows per core
NT = KR // P           # 10 key chunks
W = H + P + H          # 640 key window per query block
PEN = 30000.0
LN_EPS = 1e-5
SIM_BETA = 1.0
RADIUS = 2
F32 = mybir.dt.float32
BF16 = mybir.dt.bfloat16
AX = mybir.AxisListType
OP = mybir.AluOpType
ACT = mybir.ActivationFunctionType

LAST_EXEC_NS = None


def _bcast_ap(t, offset, step, count):
    """[128, count] AP reading the same DRAM row into every partition."""
    return bass.AP(tensor=t.tensor, offset=t.offset + offset,
                   ap=[[0, P], [step, count]])


def build_program():
    nc = bacc.Bacc("TRN2", target_bir_lowering=False, debug=False,
                   num_devices=N_CORES)
    dt_in = lambda name, shape, dt: nc.dram_tensor(name, shape, dt,
                                                   kind="ExternalInput").ap()
    xh = dt_in("xh", [KR, D], F32)
    gq = dt_in("gq", [ROWS, 2], F32)
    gk = dt_in("gk", [KR, 2], BF16)
    wq = dt_in("wq", [P, DC, D], BF16)
    wk = dt_in("wk", [P, DC, D], BF16)
    wv = dt_in("wv", [P, DC, D], BF16)
    wo = dt_in("wo", [P, DC, D], BF16)
    w1 = dt_in("w1", [P, DC, DFF], BF16)
    w2 = dt_in("w2", [P, FC, D], BF16)
    bqs = dt_in("bqs", [P, DC], F32)   # (bq * lam) chunked [p, dc]
    bks = dt_in("bks", [P, DC], F32)
    b1s = dt_in("b1s", [P, FC], F32)
    bv = dt_in("bv", [D], BF16)
    bo = dt_in("bo", [D], F32)
    b2 = dt_in("b2", [D], F32)
    l1g = dt_in("l1g", [D], BF16)
    l1b = dt_in("l1b", [D], BF16)
    l2g = dt_in("l2g", [D], BF16)
    l2b = dt_in("l2b", [D], BF16)
    out = nc.dram_tensor("out", [ROWS, D], F32, kind="ExternalOutput").ap()

    lam = float(1.0 / np.sqrt(D))

    with tile.TileContext(nc) as tc, ExitStack() as ctx:
        const = ctx.enter_context(tc.tile_pool(name="const", bufs=1))
        big = ctx.enter_context(tc.tile_pool(name="big", bufs=1))
        temps = ctx.enter_context(tc.tile_pool(name="temps", bufs=3))
        small = ctx.enter_context(tc.tile_pool(name="small", bufs=6))
        psA = ctx.enter_context(tc.tile_pool(name="psA", bufs=4, space="PSUM"))
        psS = ctx.enter_context(tc.tile_pool(name="psS", bufs=2, space="PSUM"))

        # float biases used by scalar.activation need registered const APs
        for val in (0.0, LN_EPS, 1e-16):
            ct = const.tile([P, 1], F32, tag=f"const_{val}")
            nc.vector.memset(ct, val)
            nc.const_aps.aps[(F32, val)] = ct

        # ---- constants / weights into SBUF
        wq_t = const.tile([P, DC, D], BF16)
        wk_t = const.tile([P, DC, D], BF16)
        wv_t = const.tile([P, DC, D], BF16)
        wo_t = const.tile([P, DC, D], BF16)
        w1_t = const.tile([P, DC, DFF], BF16)
        w2_t = const.tile([P, FC, D], BF16)
        for t_, a_ in [(wq_t, wq), (wk_t, wk), (wv_t, wv), (wo_t, wo),
                       (w1_t, w1), (w2_t, w2)]:
            nc.sync.dma_start(out=t_, in_=a_)
        bqs_t = const.tile([P, DC], F32)
        bks_t = const.tile([P, DC], F32)
        b1s_t = const.tile([P, FC], F32)
        for t_, a_ in [(bqs_t, bqs), (bks_t, bks), (b1s_t, b1s)]:
            nc.sync.dma_start(out=t_, in_=a_)
        bv_bc = const.tile([P, D], BF16)
        bo_bc = const.tile([P, D], F32)
        b2_bc = const.tile([P, D], F32)
        l1g_bc = const.tile([P, D], BF16)
        l1b_bc = const.tile([P, D], BF16)
        l2g_bc = const.tile([P, D], BF16)
        l2b_bc = const.tile([P, D], BF16)
        for t_, a_ in [(bv_bc, bv), (bo_bc, bo), (b2_bc, b2), (l1g_bc, l1g),
                       (l1b_bc, l1b), (l2g_bc, l2g), (l2b_bc, l2b)]:
            nc.sync.dma_start(out=t_, in_=_bcast_ap(a_, 0, 1, D))
        gxk_bc = const.tile([P, KR], BF16)
        gyk_bc = const.tile([P, KR], BF16)
        nc.sync.dma_start(out=gxk_bc, in_=_bcast_ap(gk, 0, 2, KR))
        nc.sync.dma_start(out=gyk_bc, in_=_bcast_ap(gk, 1, 2, KR))
        gq_t = const.tile([P, NB, 2], F32)
        nc.sync.dma_start(out=gq_t, in_=gq.rearrange("(nb p) c -> p nb c", p=P))
        # self-exclusion: selfpen[p, H + p] = -PEN, else 0
        selfpen = const.tile([P, W], F32)
        nc.gpsimd.memset(selfpen, -PEN)
        nc.gpsimd.affine_select(out=selfpen, in_=selfpen,
                                compare_op=OP.not_equal, fill=-2.0 * PEN,
                                base=H, channel_multiplier=1,
                                pattern=[[-1, W]])

        # ---- persistent activations
        HT = big.tile([P, DC, KR], BF16, tag="slotA")    # h^T (d-major)
        HnT = big.tile([P, DC, KR], BF16, tag="slotB")   # (h/||h||)^T
        qt = big.tile([P, DC, ROWS], BF16)  # (lam*q)^T
        kt = big.tile([P, DC, KR], BF16)    # k^T
        vt = big.tile([P, NT, D], BF16)     # v rows (row-chunk major)
        aoT = big.tile([P, DC, ROWS], BF16)
        h2T = big.tile([P, DC, ROWS], BF16)
        gt = big.tile([P, FC, ROWS], BF16, tag="slotA")  # reuses HT slot
        x2t = big.tile([P, NB, D], F32, tag="slotB")     # reuses HnT slot

        # ---- Phase A: LN1 + row stats + transposes
        for t in range(NT):
            xt = temps.tile([P, D], F32, tag="xt")
            nc.sync.dma_start(out=xt, in_=xh[t * P:(t + 1) * P, :])
            st = small.tile([P, 6], F32, tag="st")
            nc.vector.bn_stats(st, xt)
            mv = small.tile([P, 2], F32, tag="mv")
            nc.vector.bn_aggr(mv, st)
            sd = small.tile([P, 1], F32, tag="sd")
            nc.scalar.activation(sd, mv[:, 1:2], ACT.Sqrt, bias=LN_EPS)
            rstd = small.tile([P, 1], F32, tag="rstd")
            nc.vector.reciprocal(rstd, sd)
            h = temps.tile([P, D], F32, tag="h")
            nc.vector.tensor_scalar(h, xt, mv[:, 0:1], rstd,
                                    op0=OP.subtract, op1=OP.mult)
            nc.vector.tensor_tensor(h, h, l1g_bc, op=OP.mult)
            nc.vector.tensor_tensor(h, h, l1b_bc, op=OP.add)
            hb = temps.tile([P, D], BF16, tag="hb")
            nc.vector.tensor_copy(hb, h)
            scr = temps.tile([P, D], BF16, tag="scr")
            h2s = small.tile([P, 1], F32, tag="h2s")
            nc.scalar.activation(scr, h, ACT.Square, accum_out=h2s)
            nsq = small.tile([P, 1], F32, tag="nsq")
            nc.scalar.activation(nsq, h2s, ACT.Sqrt, bias=1e-16)
            rn = small.tile([P, 1], F32, tag="rn")
            nc.vector.reciprocal(rn, nsq)
            hnb = temps.tile([P, D], BF16, tag="hnb")
            nc.vector.tensor_scalar_mul(hnb, h, rn)
            for dcx in range(DC):
                nc.sync.dma_start_transpose(
                    HT[:, dcx, t * P:(t + 1) * P], hb[:, dcx * P:(dcx + 1) * P])
                nc.scalar.dma_start_transpose(
                    HnT[:, dcx, t * P:(t + 1) * P], hnb[:, dcx * P:(dcx + 1) * P])

        # ---- Phase B: projections
        # k^T (all KR rows) and q^T (own rows, scaled by lam)
        for dcx in range(DC):
            for r0, rn_ in [(0, 512), (512, 512), (1024, 256)]:
                ps = psA.tile([P, 512], F32, tag="ps")
                for ci in range(DC):
                    nc.tensor.matmul(ps[:, :rn_],
                                     wk_t[:, ci, dcx * P:(dcx + 1) * P],
                                     HT[:, ci, r0:r0 + rn_],
                                     start=(ci == 0), stop=(ci == DC - 1))
                nc.vector.tensor_scalar(kt[:, dcx, r0:r0 + rn_], ps[:, :rn_],
                                        bks_t[:, dcx:dcx + 1], None, op0=OP.add)
            for r0, rn_ in [(0, 512), (512, 256)]:
                ps = psA.tile([P, 512], F32, tag="ps")
                for ci in range(DC):
                    nc.tensor.matmul(ps[:, :rn_],
                                     wq_t[:, ci, dcx * P:(dcx + 1) * P],
                                     HT[:, ci, H + r0:H + r0 + rn_],
                                     start=(ci == 0), stop=(ci == DC - 1))
                nc.vector.tensor_scalar(qt[:, dcx, r0:r0 + rn_], ps[:, :rn_],
                                        lam, bqs_t[:, dcx:dcx + 1],
                                        op0=OP.mult, op1=OP.add)
        # v rows
        for t in range(NT):
            ps = psA.tile([P, 512], F32, tag="ps")
            for ci in range(DC):
                nc.tensor.matmul(ps, HT[:, ci, t * P:(t + 1) * P],
                                 wv_t[:, ci, :],
                                 start=(ci == 0), stop=(ci == DC - 1))
            nc.vector.tensor_tensor(vt[:, t, :], ps, bv_bc, op=OP.add)

        # ---- Phase C: band attention per 128-query block
        for b in range(NB):
            wb = P * b  # local window start (own queries sit at H + wb)
            S = psS.tile([P, W], F32, tag="S")
            for c0, cn in [(0, 512), (512, 128)]:
                for ac in range(2 * DC):
                    if ac < DC:
                        lhsT = qt[:, ac, wb:wb + P]
                        rhs = kt[:, ac, wb + c0:wb + c0 + cn]
                    else:
                        lhsT = HnT[:, ac - DC, H + wb:H + wb + P]
                        rhs = HnT[:, ac - DC, wb + c0:wb + c0 + cn]
                    nc.tensor.matmul(S[:, c0:c0 + cn], lhsT, rhs,
                                     start=(ac == 0), stop=(ac == 2 * DC - 1))
            dgx = temps.tile([P, W], BF16, tag="dgx")
            nc.vector.tensor_scalar(dgx, gxk_bc[:, wb:wb + W],
                                    gq_t[:, b, 0:1], None, op0=OP.subtract)
            dgy = temps.tile([P, W], BF16, tag="dgy")
            nc.vector.tensor_scalar(dgy, gyk_bc[:, wb:wb + W],
                                    gq_t[:, b, 1:2], None, op0=OP.subtract)
            # valid iff max(dgx,dgy) <= R+0.5 and min(dgx,dgy) >= -(R+0.5)
            mn = temps.tile([P, W], BF16, tag="ind")
            nc.vector.tensor_tensor(mn, dgx, dgy, op=OP.min)
            nc.vector.tensor_tensor(dgx, dgx, dgy, op=OP.max)
            nc.vector.tensor_scalar(dgy, dgx, RADIUS + 0.5, None, op0=OP.is_le)
            vcnt = small.tile([P, 1], F32, tag="vcnt")
            nc.vector.scalar_tensor_tensor(mn, mn, -(RADIUS + 0.5), dgy,
                                           op0=OP.is_ge, op1=OP.mult,
                                           accum_out=vcnt)
            # S += PEN*valid - PEN (constant -PEN baked into selfpen tile)
            nc.vector.scalar_tensor_tensor(S, mn, PEN, S,
                                           op0=OP.mult, op1=OP.add)
            nc.vector.tensor_tensor(S, S, selfpen, op=OP.add)
            m_ = small.tile([P, 1], F32, tag="m_")
            nc.vector.tensor_reduce(m_, S, axis=AX.X, op=OP.max)
            negm = small.tile([P, 1], F32, tag="negm")
            nc.vector.tensor_scalar(negm, m_, -1.0, None, op0=OP.mult)
            pb = temps.tile([P, W], BF16, tag="pb")
            srow = small.tile([P, 1], F32, tag="srow")
            nc.scalar.activation(pb, S, ACT.Exp, bias=negm, scale=1.0,
                                 accum_out=srow)
            rs = small.tile([P, 1], F32, tag="rs")
            nc.vector.reciprocal(rs, srow)
            PT = temps.tile([P, W // P, P], BF16, tag="PT")
            for j in range(W // P):
                nc.sync.dma_start_transpose(PT[:, j, :], pb[:, j * P:(j + 1) * P])
            po = psA.tile([P, 512], F32, tag="ps")
            for j in range(W // P):
                nc.tensor.matmul(po, PT[:, j, :], vt[:, b + j, :],
                                 start=(j == 0), stop=(j == W // P - 1))
            has = small.tile([P, 1], F32, tag="has")
            nc.vector.tensor_scalar(has, vcnt, 1.5, None, op0=OP.is_ge)
            o1 = temps.tile([P, D], F32, tag="o1")
            nc.vector.tensor_scalar_mul(o1, po, rs)
            nc.vector.tensor_tensor(o1, o1, vt[:, H // P + b, :], op=OP.subtract)
            aob = temps.tile([P, D], BF16, tag="aob")
            nc.vector.scalar_tensor_tensor(aob, o1, has, vt[:, H // P + b, :],
                                           op0=OP.mult, op1=OP.add)
            for dcx in range(DC):
                nc.scalar.dma_start_transpose(
                    aoT[:, dcx, b * P:(b + 1) * P], aob[:, dcx * P:(dcx + 1) * P])

        # ---- Phase D: residual + Wo, LN2
        for b in range(NB):
            ps = psA.tile([P, 512], F32, tag="ps")
            for ci in range(DC):
                nc.tensor.matmul(ps, aoT[:, ci, b * P:(b + 1) * P],
                                 wo_t[:, ci, :],
                                 start=(ci == 0), stop=(ci == DC - 1))
            xt = temps.tile([P, D], F32, tag="xt2")
            nc.sync.dma_start(out=xt, in_=xh[H + b * P:H + (b + 1) * P, :])
            x2 = x2t[:, b, :]
            nc.vector.tensor_tensor(x2, ps, xt, op=OP.add)
            nc.vector.tensor_tensor(x2, x2, bo_bc, op=OP.add)
            st = small.tile([P, 6], F32, tag="st")
            nc.vector.bn_stats(st, x2)
            mv = small.tile([P, 2], F32, tag="mv")
            nc.vector.bn_aggr(mv, st)
            sd = small.tile([P, 1], F32, tag="sd")
            nc.scalar.activation(sd, mv[:, 1:2], ACT.Sqrt, bias=LN_EPS)
            rstd = small.tile([P, 1], F32, tag="rstd")
            nc.vector.reciprocal(rstd, sd)
            h2 = temps.tile([P, D], F32, tag="h")
            nc.vector.tensor_scalar(h2, x2, mv[:, 0:1], rstd,
                                    op0=OP.subtract, op1=OP.mult)
            nc.vector.tensor_tensor(h2, h2, l2g_bc, op=OP.mult)
            nc.vector.tensor_tensor(h2, h2, l2b_bc, op=OP.add)
            h2b = temps.tile([P, D], BF16, tag="hb")
            nc.vector.tensor_copy(h2b, h2)
            for dcx in range(DC):
                nc.sync.dma_start_transpose(
                    h2T[:, dcx, b * P:(b + 1) * P], h2b[:, dcx * P:(dcx + 1) * P])

        # ---- Phase E: FFN
        GR = 384  # row group for a^T matmuls (fits one PSUM bank)
        for g0 in range(0, ROWS, GR):
            for fcx in range(FC):
                ps = psA.tile([P, 512], F32, tag="ps")
                for ci in range(DC):
                    nc.tensor.matmul(ps[:, :GR],
                                     w1_t[:, ci, fcx * P:(fcx + 1) * P],
                                     h2T[:, ci, g0:g0 + GR],
                                     start=(ci == 0), stop=(ci == DC - 1))
                nc.scalar.activation(gt[:, fcx, g0:g0 + GR], ps[:, :GR],
                                     ACT.Gelu, bias=b1s_t[:, fcx:fcx + 1],
                                     scale=1.0)
        for b in range(NB):
            ps = psA.tile([P, 512], F32, tag="ps")
            for fcx in range(FC):
                nc.tensor.matmul(ps, gt[:, fcx, b * P:(b + 1) * P],
                                 w2_t[:, fcx, :],
                                 start=(fcx == 0), stop=(fcx == FC - 1))
            fo = temps.tile([P, D], F32, tag="fo")
            nc.vector.tensor_tensor(fo, ps, x2t[:, b, :], op=OP.add)
            nc.vector.tensor_tensor(fo, fo, b2_bc, op=OP.add)
            nc.sync.dma_start(out=out[b * P:(b + 1) * P, :], in_=fo)

    nc.compile()
    return nc


_prog = None


def _get_program():
    global _prog
    if _prog is None:
        _prog = build_program()
    return _prog


def _np_fallback(x, grid, Wq, bq, Wk, bk, Wv, bv, Wo, bo,
                 ln1_g, ln1_b, ln2_g, ln2_b, W1, b1, W2, b2):
    """Exact fp64 host path (only used if an input violates the band bound)."""
    from scipy.special import erf
    x = np.asarray(x, np.float64)
    g = np.asarray(grid).astype(np.float64)

    def ln(v, gm, bt, eps=1e-5):
        mu = v.mean(-1, keepdims=True)
        var = v.var(-1, keepdims=True)
        return (v - mu) / np.sqrt(var + eps) * gm + bt

    h = ln(x, ln1_g, ln1_b)
    q = h @ Wq + bq
    k = h @ Wk + bk
    v = h @ Wv + bv
    hn = h / np.maximum(np.linalg.norm(h, axis=-1, keepdims=True), 1e-8)
    scale = 1.0 / np.sqrt(D)
    n = x.shape[0]
    outp = np.empty_like(x)
    for s in range(0, n, 512):
        e = min(s + 512, n)
        dx = np.abs(g[s:e, None, 0] - g[None, :, 0])
        dy = np.abs(g[s:e, None, 1] - g[None, :, 1])
        mask = (dx <= RADIUS) & (dy <= RADIUS)
        mask[np.arange(e - s), np.arange(s, e)] = False
        logits = (q[s:e] @ k.T) * scale + SIM_BETA * (hn[s:e] @ hn.T)
        logits = np.where(mask, logits, -1e30)
        m = logits.max(-1, keepdims=True)
        p = np.exp(logits - m)
        att = p / p.sum(-1, keepdims=True)
        o = att @ v
        outp[s:e] = np.where(mask.any(1, keepdims=True), o, v[s:e])
    x = x + outp @ Wo + bo
    h2 = ln(x, ln2_g, ln2_b)
    a = h2 @ W1 + b1
    gelu = 0.5 * a * (1.0 + erf(a / np.sqrt(2.0)))
    return (x + gelu @ W2 + b2).astype(np.float32)


def kernel(x, grid, Wq, bq, Wk, bk, Wv, bv, Wo, bo,
           ln1_g, ln1_b, ln2_g, ln2_b, W1, b1, W2, b2):
    global LAST_EXEC_NS
    x = np.ascontiguousarray(np.asarray(x, np.float32))
    g = np.asarray(grid).astype(np.int64)

    # ---- host-side sharding: sort rows by (gx, gy)
    perm = np.lexsort((g[:, 1], g[:, 0]))
    inv_perm = np.argsort(perm)
    gs = g[perm]
    xs = x[perm]

    # band-coverage check: all neighbours of block b within [128b-H, 128b+128+H)
    ok = True
    gx = gs[:, 0]
    for b in range(N // P):
        lo = np.searchsorted(gx, gx[b * P:(b + 1) * P].min() - RADIUS, "left")
        hi = np.searchsorted(gx, gx[b * P:(b + 1) * P].max() + RADIUS, "right")
        if b * P - lo > H or hi - (b * P + P) > H:
            ok = False
            break
    if not ok:
        return _np_fallback(x, grid, Wq, bq, Wk, bk, Wv, bv, Wo, bo,
                            ln1_g, ln1_b, ln2_g, ln2_b, W1, b1, W2, b2)

    lam = 1.0 / np.sqrt(D)
    b16 = lambda a: np.ascontiguousarray(np.asarray(a)).astype(ml_dtypes.bfloat16)
    f32 = lambda a: np.ascontiguousarray(np.asarray(a, np.float32))

    # weights in [p, chunk, out] layout (p = contraction index % 128)
    chunked = lambda w_, nch: np.ascontiguousarray(
        b16(w_).reshape(nch, P, -1).transpose(1, 0, 2))
    wq_h, wk_h, wv_h, wo_h = (chunked(w_, DC) for w_ in (Wq, Wk, Wv, Wo))
    w1_h = chunked(W1, DC)
    w2_h = chunked(W2, FC)
    bqs_h = np.ascontiguousarray((np.asarray(bq, np.float64) * lam)
                                 .astype(np.float32).reshape(DC, P).T)
    bks_h = np.ascontiguousarray(f32(bk).reshape(DC, P).T)
    b1s_h = np.ascontiguousarray(f32(b1).reshape(FC, P).T)

    shared = dict(wq=wq_h, wk=wk_h, wv=wv_h, wo=wo_h, w1=w1_h, w2=w2_h,
                  bqs=bqs_h, bks=bks_h, b1s=b1s_h,
                  bv=b16(bv), bo=f32(bo), b2=f32(b2),
                  l1g=b16(ln1_g), l1b=b16(ln1_b),
                  l2g=b16(ln2_g), l2b=b16(ln2_b))

    in_maps = []
    gs_f = gs.astype(np.float32)
    for c in range(N_CORES):
        glo = c * ROWS - H
        xh_c = np.zeros((KR, D), np.float32)
        gk_c = np.full((KR, 2), 10000.0, np.float32)
        s0, s1 = max(0, glo), min(N, glo + KR)
        xh_c[s0 - glo:s1 - glo] = xs[s0:s1]
        gk_c[s0 - glo:s1 - glo] = gs_f[s0:s1]
        in_maps.append(dict(shared,
                            xh=xh_c,
                            gk=gk_c.astype(ml_dtypes.bfloat16),
                            gq=gs_f[c * ROWS:(c + 1) * ROWS]))

    nc = _get_program()
    tmpdir = os.environ.get("KERNEL_TRACE_DIR") or None
    res = run_bass_kernel_spmd(nc, in_maps, list(range(N_CORES)),
                               tmpdir=tmpdir)
    LAST_EXEC_NS = res.exec_time_ns
    out_sorted = np.concatenate([res.results[c]["out"]
                                 for c in range(N_CORES)], axis=0)
    return np.ascontiguousarray(out_sorted[inv_perm]).astype(np.float32)


# revision 10
# speedup vs baseline: 2.6856x; 2.6856x over previous
"""LocalBandSimilarityBlock — Trainium2 Bass kernel, 8-way sequence-parallel.

Strategy: sort nodes by grid-x (host-side sharding permutation). After the
sort, every node's radius-2 neighbourhood lies within a +-256-row band in
sorted order (verified per-input on host). Each of the 8 cores owns 768
sorted query rows plus a 256-row halo on each side (1280 key rows total,
zero-padded at the edges) — so cores run fully independently, no
collectives. Per 128-query block, attention is computed against a 640-row
key window instead of all 6144 keys (12x less tensor work than dense).

On-device math (per core): LN1 -> q/k/v projections (bf16 matmuls, fp32
accum) -> band-masked augmented-logit attention (q.k/sqrt(D) + cos-sim via
concatenated [q | hn] contraction) -> fp32 softmax with isolated-node
fallback to v -> residual + Wo -> LN2 -> FFN(gelu) -> residual. The grid
band mask (Chebyshev distance <= 2, self excluded) is computed on device
from the grid coordinates.
"""
import os
import sys

import numpy as np

sys.path.insert(0, "/opt/trn_rl_repo")

import ml_dtypes  # noqa: E402
from contextlib import ExitStack  # noqa: E402

import concourse.bacc as bacc  # noqa: E402
import concourse.bass as bass  # noqa: E402
import concourse.tile as tile  # noqa: E402
from concourse import mybir  # noqa: E402
from concourse.bass_utils import run_bass_kernel_spmd  # noqa: E402

P = 128
D = 512
DC = D // P            # 4 d-chunks
DFF = 2048
FC = DFF // P          # 16 ffn chunks
N = 6144
N_CORES = 8
ROWS = N // N_CORES    # 768 query rows per core
NB = ROWS // P         # 6 query blocks per core
H = 256                # halo rows each side (multiple of 128)
KR = ROWS + 2 * H      # 1280 key rows per core
NT = KR // P           # 10 key chunks
W = H + P + H          # 640 key window per query block
PEN = 30000.0
LN_EPS = 1e-5
SIM_BETA = 1.0
RADIUS = 2
F32 = mybir.dt.float32
BF16 = mybir.dt.bfloat16
AX = mybir.AxisListType
OP = mybir.AluOpType
ACT = mybir.ActivationFunctionType

LAST_EXEC_NS = None


def _bcast_ap(t, offset, step, count):
    """[128, count] AP reading the same DRAM row into every partition."""
    return bass.AP(tensor=t.tensor, offset=t.offset + offset,
                   ap=[[0, P], [step, count]])


def build_program():
    nc = bacc.Bacc("TRN2", target_bir_lowering=False, debug=False,
                   num_devices=N_CORES)
    dt_in = lambda name, shape, dt: nc.dram_tensor(name, shape, dt,
                                                   kind="ExternalInput").ap()
    xh = dt_in("xh", [KR, D], F32)
    gq = dt_in("gq", [ROWS, 2], F32)
    gxkb = dt_in("gxkb", [P, KR], BF16)
    gykb = dt_in("gykb", [P, KR], BF16)
    wq = dt_in("wq", [P, DC, D], BF16)
    wk = dt_in("wk", [P, DC, D], BF16)
    wv = dt_in("wv", [P, DC, D], BF16)
    wo = dt_in("wo", [P, DC, D], BF16)
    w1 = dt_in("w1", [P, DC, DFF], BF16)
    w2 = dt_in("w2", [P, FC, D], BF16)
    bqs = dt_in("bqs", [P, DC], F32)   # (bq * lam) chunked [p, dc]
    bks = dt_in("bks", [P, DC], F32)
    b1s = dt_in("b1s", [P, FC], F32)
    bv = dt_in("bv", [P, D], BF16)
    bo = dt_in("bo", [P, D], F32)
    b2 = dt_in("b2", [P, D], F32)
    l1g = dt_in("l1g", [P, D], BF16)
    l1b = dt_in("l1b", [P, D], BF16)
    l2g = dt_in("l2g", [P, D], BF16)
    l2b = dt_in("l2b", [P, D], BF16)
    out = nc.dram_tensor("out", [ROWS, D], F32, kind="ExternalOutput").ap()

    lam = float(1.0 / np.sqrt(D))

    with tile.TileContext(nc) as tc, ExitStack() as ctx:
        const = ctx.enter_context(tc.tile_pool(name="const", bufs=1))
        big = ctx.enter_context(tc.tile_pool(name="big", bufs=1))
        temps = ctx.enter_context(tc.tile_pool(name="temps", bufs=3))
        small = ctx.enter_context(tc.tile_pool(name="small", bufs=6))
        psA = ctx.enter_context(tc.tile_pool(name="psA", bufs=4, space="PSUM"))
        psS = ctx.enter_context(tc.tile_pool(name="psS", bufs=2, space="PSUM"))

        # float biases used by scalar.activation need registered const APs
        for val in (0.0, LN_EPS, 1e-16):
            ct = const.tile([P, 1], F32, tag=f"const_{val}")
            nc.vector.memset(ct, val)
            nc.const_aps.aps[(F32, val)] = ct

        # ---- constants / weights into SBUF
        wq_t = const.tile([P, DC, D], BF16)
        wk_t = const.tile([P, DC, D], BF16)
        wv_t = const.tile([P, DC, D], BF16)
        wo_t = const.tile([P, DC, D], BF16)
        w1_t = const.tile([P, DC, DFF], BF16)
        w2_t = const.tile([P, FC, D], BF16)
        for t_, a_ in [(wq_t, wq), (wk_t, wk), (wv_t, wv)]:
            nc.sync.dma_start(out=t_, in_=a_)
        bqs_t = const.tile([P, DC], F32)
        bks_t = const.tile([P, DC], F32)
        b1s_t = const.tile([P, FC], F32)
        for t_, a_ in [(bqs_t, bqs), (bks_t, bks), (b1s_t, b1s)]:
            nc.sync.dma_start(out=t_, in_=a_)
        bv_bc = const.tile([P, D], BF16)
        bo_bc = const.tile([P, D], F32)
        b2_bc = const.tile([P, D], F32)
        l1g_bc = const.tile([P, D], BF16)
        l1b_bc = const.tile([P, D], BF16)
        l2g_bc = const.tile([P, D], BF16)
        l2b_bc = const.tile([P, D], BF16)
        for t_, a_ in [(bv_bc, bv), (bo_bc, bo), (b2_bc, b2), (l1g_bc, l1g),
                       (l1b_bc, l1b), (l2g_bc, l2g), (l2b_bc, l2b)]:
            nc.scalar.dma_start(out=t_, in_=a_)
        gxk_bc = const.tile([P, KR], BF16)
        gyk_bc = const.tile([P, KR], BF16)
        nc.scalar.dma_start(out=gxk_bc, in_=gxkb)
        nc.scalar.dma_start(out=gyk_bc, in_=gykb)
        gq_t = const.tile([P, NB, 2], F32)
        nc.sync.dma_start(out=gq_t, in_=gq.rearrange("(nb p) c -> p nb c", p=P))
        # self-exclusion: selfpen[p, H + p] = -PEN, else 0
        selfpen = const.tile([P, W], F32)
        nc.gpsimd.memset(selfpen, -PEN)
        nc.gpsimd.affine_select(out=selfpen, in_=selfpen,
                                compare_op=OP.not_equal, fill=-2.0 * PEN,
                                base=H, channel_multiplier=1,
                                pattern=[[-1, W]])

        # ---- persistent activations
        HT = big.tile([P, DC, KR], BF16, tag="slotA")    # h^T (d-major)
        HnT = big.tile([P, DC, KR], BF16, tag="slotB")   # (h/||h||)^T
        qt = big.tile([P, DC, ROWS], BF16)  # (lam*q)^T
        kt = big.tile([P, DC, KR], BF16)    # k^T
        vt = big.tile([P, NT, D], BF16)     # v rows (row-chunk major)
        aoT = big.tile([P, DC, ROWS], BF16)
        h2T = big.tile([P, DC, ROWS], BF16)
        gt = big.tile([P, FC, ROWS], BF16, tag="slotA")  # reuses HT slot
        x2t = big.tile([P, NB, D], F32, tag="slotB")     # reuses HnT slot

        # ---- Phase A: LN1 + row stats + transposes
        for t in range(NT):
            xt = temps.tile([P, D], F32, tag="xt")
            nc.sync.dma_start(out=xt, in_=xh[t * P:(t + 1) * P, :])
            st = small.tile([P, 6], F32, tag="st")
            nc.vector.bn_stats(st, xt)
            mv = small.tile([P, 2], F32, tag="mv")
            nc.vector.bn_aggr(mv, st)
            sd = small.tile([P, 1], F32, tag="sd")
            nc.scalar.activation(sd, mv[:, 1:2], ACT.Sqrt, bias=LN_EPS)
            rstd = small.tile([P, 1], F32, tag="rstd")
            nc.vector.reciprocal(rstd, sd)
            h = temps.tile([P, D], F32, tag="h")
            nc.vector.tensor_scalar(h, xt, mv[:, 0:1], rstd,
                                    op0=OP.subtract, op1=OP.mult)
            nc.vector.tensor_tensor(h, h, l1g_bc, op=OP.mult)
            nc.vector.tensor_tensor(h, h, l1b_bc, op=OP.add)
            hb = temps.tile([P, D], BF16, tag="hb")
            nc.vector.tensor_copy(hb, h)
            scr = temps.tile([P, D], BF16, tag="scr")
            h2s = small.tile([P, 1], F32, tag="h2s")
            nc.scalar.activation(scr, h, ACT.Square, accum_out=h2s)
            nsq = small.tile([P, 1], F32, tag="nsq")
            nc.scalar.activation(nsq, h2s, ACT.Sqrt, bias=1e-16)
            rn = small.tile([P, 1], F32, tag="rn")
            nc.vector.reciprocal(rn, nsq)
            hnb = temps.tile([P, D], BF16, tag="hnb")
            nc.vector.tensor_scalar_mul(hnb, h, rn)
            nc.sync.dma_start_transpose(HT[:, :, t * P:(t + 1) * P], hb)
            nc.scalar.dma_start_transpose(HnT[:, :, t * P:(t + 1) * P], hnb)

        # ---- Phase B: projections
        # k^T (all KR rows) and q^T (own rows, scaled by lam)
        for dcx in range(DC):
            for r0, rn_ in [(0, 512), (512, 512), (1024, 256)]:
                ps = psA.tile([P, 512], F32, tag="ps")
                for ci in range(DC):
                    nc.tensor.matmul(ps[:, :rn_],
                                     wk_t[:, ci, dcx * P:(dcx + 1) * P],
                                     HT[:, ci, r0:r0 + rn_],
                                     start=(ci == 0), stop=(ci == DC - 1))
                nc.vector.tensor_scalar(kt[:, dcx, r0:r0 + rn_], ps[:, :rn_],
                                        bks_t[:, dcx:dcx + 1], None, op0=OP.add)
            for r0, rn_ in [(0, 512), (512, 256)]:
                ps = psA.tile([P, 512], F32, tag="ps")
                for ci in range(DC):
                    nc.tensor.matmul(ps[:, :rn_],
                                     wq_t[:, ci, dcx * P:(dcx + 1) * P],
                                     HT[:, ci, H + r0:H + r0 + rn_],
                                     start=(ci == 0), stop=(ci == DC - 1))
                nc.vector.tensor_scalar(qt[:, dcx, r0:r0 + rn_], ps[:, :rn_],
                                        lam, bqs_t[:, dcx:dcx + 1],
                                        op0=OP.mult, op1=OP.add)
        # v rows
        for t in range(NT):
            ps = psA.tile([P, 512], F32, tag="ps")
            for ci in range(DC):
                nc.tensor.matmul(ps, HT[:, ci, t * P:(t + 1) * P],
                                 wv_t[:, ci, :],
                                 start=(ci == 0), stop=(ci == DC - 1))
            nc.vector.tensor_tensor(vt[:, t, :], ps, bv_bc, op=OP.add)

        # ---- Phase C: band attention per 128-query block
        for b in range(NB):
            wb = P * b  # local window start (own queries sit at H + wb)
            S = psS.tile([P, W], F32, tag="S")
            for c0, cn in [(0, 512), (512, 128)]:
                for ac in range(2 * DC):
                    if ac < DC:
                        lhsT = qt[:, ac, wb:wb + P]
                        rhs = kt[:, ac, wb + c0:wb + c0 + cn]
                    else:
                        lhsT = HnT[:, ac - DC, H + wb:H + wb + P]
                        rhs = HnT[:, ac - DC, wb + c0:wb + c0 + cn]
                    nc.tensor.matmul(S[:, c0:c0 + cn], lhsT, rhs,
                                     start=(ac == 0), stop=(ac == 2 * DC - 1))
            dgx = temps.tile([P, W], BF16, tag="dgx")
            nc.vector.tensor_scalar(dgx, gxk_bc[:, wb:wb + W],
                                    gq_t[:, b, 0:1], None, op0=OP.subtract)
            dgy = temps.tile([P, W], BF16, tag="dgy")
            nc.vector.tensor_scalar(dgy, gyk_bc[:, wb:wb + W],
                                    gq_t[:, b, 1:2], None, op0=OP.subtract)
            # valid iff max(dgx,dgy) <= R+0.5 and min(dgx,dgy) >= -(R+0.5)
            mn = temps.tile([P, W], BF16, tag="ind")
            nc.vector.tensor_tensor(mn, dgx, dgy, op=OP.min)
            nc.vector.tensor_tensor(dgx, dgx, dgy, op=OP.max)
            nc.vector.tensor_scalar(dgy, dgx, RADIUS + 0.5, None, op0=OP.is_le)
            vcnt = small.tile([P, 1], F32, tag="vcnt")
            nc.vector.scalar_tensor_tensor(mn, mn, -(RADIUS + 0.5), dgy,
                                           op0=OP.is_ge, op1=OP.mult,
                                           accum_out=vcnt)
            # S += PEN*valid - PEN (constant -PEN baked into selfpen tile)
            nc.vector.scalar_tensor_tensor(S, mn, PEN, S,
                                           op0=OP.mult, op1=OP.add)
            nc.vector.tensor_tensor(S, S, selfpen, op=OP.add)
            m_ = small.tile([P, 1], F32, tag="m_")
            nc.vector.tensor_reduce(m_, S, axis=AX.X, op=OP.max)
            negm = small.tile([P, 1], F32, tag="negm")
            nc.vector.tensor_scalar(negm, m_, -1.0, None, op0=OP.mult)
            pb = temps.tile([P, W], BF16, tag="pb")
            srow = small.tile([P, 1], F32, tag="srow")
            nc.scalar.activation(pb, S, ACT.Exp, bias=negm, scale=1.0,
                                 accum_out=srow)
            rs = small.tile([P, 1], F32, tag="rs")
            nc.vector.reciprocal(rs, srow)
            PT = temps.tile([P, W // P, P], BF16, tag="PT")
            nc.sync.dma_start_transpose(PT[:, :, :], pb)
            po = psA.tile([P, 512], F32, tag="ps")
            for j in range(W // P):
                nc.tensor.matmul(po, PT[:, j, :], vt[:, b + j, :],
                                 start=(j == 0), stop=(j == W // P - 1))
            has = small.tile([P, 1], F32, tag="has")
            nc.vector.tensor_scalar(has, vcnt, 1.5, None, op0=OP.is_ge)
            o1 = temps.tile([P, D], F32, tag="o1")
            nc.vector.tensor_scalar_mul(o1, po, rs)
            nc.vector.tensor_tensor(o1, o1, vt[:, H // P + b, :], op=OP.subtract)
            aob = temps.tile([P, D], BF16, tag="aob")
            nc.vector.scalar_tensor_tensor(aob, o1, has, vt[:, H // P + b, :],
                                           op0=OP.mult, op1=OP.add)
            nc.scalar.dma_start_transpose(aoT[:, :, b * P:(b + 1) * P], aob)

        # ---- Phase D: residual + Wo, LN2
        nc.sync.dma_start(out=wo_t, in_=wo)
        nc.scalar.dma_start(out=w1_t, in_=w1)
        nc.sync.dma_start(out=w2_t, in_=w2)
        for b in range(NB):
            ps = psA.tile([P, 512], F32, tag="ps")
            for ci in range(DC):
                nc.tensor.matmul(ps, aoT[:, ci, b * P:(b + 1) * P],
                                 wo_t[:, ci, :],
                                 start=(ci == 0), stop=(ci == DC - 1))
            xt = temps.tile([P, D], F32, tag="xt2")
            nc.gpsimd.dma_start(out=xt, in_=xh[H + b * P:H + (b + 1) * P, :])
            x2 = x2t[:, b, :]
            nc.vector.tensor_tensor(x2, ps, xt, op=OP.add)
            nc.vector.tensor_tensor(x2, x2, bo_bc, op=OP.add)
            st = small.tile([P, 6], F32, tag="st")
            nc.vector.bn_stats(st, x2)
            mv = small.tile([P, 2], F32, tag="mv")
            nc.vector.bn_aggr(mv, st)
            sd = small.tile([P, 1], F32, tag="sd")
            nc.scalar.activation(sd, mv[:, 1:2], ACT.Sqrt, bias=LN_EPS)
            rstd = small.tile([P, 1], F32, tag="rstd")
            nc.vector.reciprocal(rstd, sd)
            h2 = temps.tile([P, D], F32, tag="h")
            nc.vector.tensor_scalar(h2, x2, mv[:, 0:1], rstd,
                                    op0=OP.subtract, op1=OP.mult)
            nc.vector.tensor_tensor(h2, h2, l2g_bc, op=OP.mult)
            nc.vector.tensor_tensor(h2, h2, l2b_bc, op=OP.add)
            h2b = temps.tile([P, D], BF16, tag="hb")
            nc.vector.tensor_copy(h2b, h2)
            nc.sync.dma_start_transpose(h2T[:, :, b * P:(b + 1) * P], h2b)

        # ---- Phase E: FFN
        GR = 384  # row group for a^T matmuls (fits one PSUM bank)
        for g0 in range(0, ROWS, GR):
            for fcx in range(FC):
                ps = psA.tile([P, 512], F32, tag="ps")
                for ci in range(DC):
                    nc.tensor.matmul(ps[:, :GR],
                                     w1_t[:, ci, fcx * P:(fcx + 1) * P],
                                     h2T[:, ci, g0:g0 + GR],
                                     start=(ci == 0), stop=(ci == DC - 1))
                nc.scalar.activation(gt[:, fcx, g0:g0 + GR], ps[:, :GR],
                                     ACT.Gelu, bias=b1s_t[:, fcx:fcx + 1],
                                     scale=1.0)
        for b in range(NB):
            ps = psA.tile([P, 512], F32, tag="ps")
            for fcx in range(FC):
                nc.tensor.matmul(ps, gt[:, fcx, b * P:(b + 1) * P],
                                 w2_t[:, fcx, :],
                                 start=(fcx == 0), stop=(fcx == FC - 1))
            fo = temps.tile([P, D], F32, tag="fo")
            nc.vector.tensor_tensor(fo, ps, x2t[:, b, :], op=OP.add)
            nc.vector.tensor_tensor(fo, fo, b2_bc, op=OP.add)
            nc.sync.dma_start(out=out[b * P:(b + 1) * P, :], in_=fo)

    nc.compile()
    return nc


_prog = None


def _get_program():
    global _prog
    if _prog is None:
        _prog = build_program()
    return _prog


def _np_fallback(x, grid, Wq, bq, Wk, bk, Wv, bv, Wo, bo,
                 ln1_g, ln1_b, ln2_g, ln2_b, W1, b1, W2, b2):
    """Exact fp64 host path (only used if an input violates the band bound)."""
    from scipy.special import erf
    x = np.asarray(x, np.float64)
    g = np.asarray(grid).astype(np.float64)

    def ln(v, gm, bt, eps=1e-5):
        mu = v.mean(-1, keepdims=True)
        var = v.var(-1, keepdims=True)
        return (v - mu) / np.sqrt(var + eps) * gm + bt

    h = ln(x, ln1_g, ln1_b)
    q = h @ Wq + bq
    k = h @ Wk + bk
    v = h @ Wv + bv
    hn = h / np.maximum(np.linalg.norm(h, axis=-1, keepdims=True), 1e-8)
    scale = 1.0 / np.sqrt(D)
    n = x.shape[0]
    outp = np.empty_like(x)
    for s in range(0, n, 512):
        e = min(s + 512, n)
        dx = np.abs(g[s:e, None, 0] - g[None, :, 0])
        dy = np.abs(g[s:e, None, 1] - g[None, :, 1])
        mask = (dx <= RADIUS) & (dy <= RADIUS)
        mask[np.arange(e - s), np.arange(s, e)] = False
        logits = (q[s:e] @ k.T) * scale + SIM_BETA * (hn[s:e] @ hn.T)
        logits = np.where(mask, logits, -1e30)
        m = logits.max(-1, keepdims=True)
        p = np.exp(logits - m)
        att = p / p.sum(-1, keepdims=True)
        o = att @ v
        outp[s:e] = np.where(mask.any(1, keepdims=True), o, v[s:e])
    x = x + outp @ Wo + bo
    h2 = ln(x, ln2_g, ln2_b)
    a = h2 @ W1 + b1
    gelu = 0.5 * a * (1.0 + erf(a / np.sqrt(2.0)))
    return (x + gelu @ W2 + b2).astype(np.float32)


def kernel(x, grid, Wq, bq, Wk, bk, Wv, bv, Wo, bo,
           ln1_g, ln1_b, ln2_g, ln2_b, W1, b1, W2, b2):
    global LAST_EXEC_NS
    x = np.ascontiguousarray(np.asarray(x, np.float32))
    g = np.asarray(grid).astype(np.int64)

    # ---- host-side sharding: sort rows by (gx, gy)
    perm = np.lexsort((g[:, 1], g[:, 0]))
    inv_perm = np.argsort(perm)
    gs = g[perm]
    xs = x[perm]

    # band-coverage check: all neighbours of block b within [128b-H, 128b+128+H)
    ok = True
    gx = gs[:, 0]
    for b in range(N // P):
        lo = np.searchsorted(gx, gx[b * P:(b + 1) * P].min() - RADIUS, "left")
        hi = np.searchsorted(gx, gx[b * P:(b + 1) * P].max() + RADIUS, "right")
        if b * P - lo > H or hi - (b * P + P) > H:
            ok = False
            break
    if not ok:
        return _np_fallback(x, grid, Wq, bq, Wk, bk, Wv, bv, Wo, bo,
                            ln1_g, ln1_b, ln2_g, ln2_b, W1, b1, W2, b2)

    lam = 1.0 / np.sqrt(D)
    b16 = lambda a: np.ascontiguousarray(np.asarray(a)).astype(ml_dtypes.bfloat16)
    f32 = lambda a: np.ascontiguousarray(np.asarray(a, np.float32))

    # weights in [p, chunk, out] layout (p = contraction index % 128)
    chunked = lambda w_, nch: np.ascontiguousarray(
        b16(w_).reshape(nch, P, -1).transpose(1, 0, 2))
    wq_h, wk_h, wv_h, wo_h = (chunked(w_, DC) for w_ in (Wq, Wk, Wv, Wo))
    w1_h = chunked(W1, DC)
    w2_h = chunked(W2, FC)
    bqs_h = np.ascontiguousarray((np.asarray(bq, np.float64) * lam)
                                 .astype(np.float32).reshape(DC, P).T)
    bks_h = np.ascontiguousarray(f32(bk).reshape(DC, P).T)
    b1s_h = np.ascontiguousarray(f32(b1).reshape(FC, P).T)

    bcast = lambda a: np.ascontiguousarray(np.broadcast_to(a, (P, D)))
    shared = dict(wq=wq_h, wk=wk_h, wv=wv_h, wo=wo_h, w1=w1_h, w2=w2_h,
                  bqs=bqs_h, bks=bks_h, b1s=b1s_h,
                  bv=bcast(b16(bv)), bo=bcast(f32(bo)), b2=bcast(f32(b2)),
                  l1g=bcast(b16(ln1_g)), l1b=bcast(b16(ln1_b)),
                  l2g=bcast(b16(ln2_g)), l2b=bcast(b16(ln2_b)))

    in_maps = []
    gs_f = gs.astype(np.float32)
    for c in range(N_CORES):
        glo = c * ROWS - H
        xh_c = np.zeros((KR, D), np.float32)
        gk_c = np.full((KR, 2), 10000.0, np.float32)
        s0, s1 = max(0, glo), min(N, glo + KR)
        xh_c[s0 - glo:s1 - glo] = xs[s0:s1]
        gk_c[s0 - glo:s1 - glo] = gs_f[s0:s1]
        gkb = gk_c.astype(ml_dtypes.bfloat16)
        in_maps.append(dict(shared,
                            xh=xh_c,
                            gxkb=np.ascontiguousarray(
                                np.broadcast_to(gkb[:, 0], (P, KR))),
                            gykb=np.ascontiguousarray(
                                np.broadcast_to(gkb[:, 1], (P, KR))),
                            gq=gs_f[c * ROWS:(c + 1) * ROWS]))

    nc = _get_program()
    tmpdir = os.environ.get("KERNEL_TRACE_DIR") or None
    res = run_bass_kernel_spmd(nc, in_maps, list(range(N_CORES)),
                               tmpdir=tmpdir)
    LAST_EXEC_NS = res.exec_time_ns
    out_sorted = np.concatenate([res.results[c]["out"]
                                 for c in range(N_CORES)], axis=0)
    return np.ascontiguousarray(out_sorted[inv_perm]).astype(np.float32)


# revision 15
# speedup vs baseline: 3.0540x; 1.1372x over previous
"""LocalBandSimilarityBlock — Trainium2 Bass kernel, 8-way sequence-parallel.

Strategy: sort nodes by grid-x (host-side sharding permutation). After the
sort, every node's radius-2 neighbourhood lies within a +-256-row band in
sorted order (verified per-input on host). Each of the 8 cores owns 768
sorted query rows plus a 256-row halo on each side (1280 key rows total,
zero-padded at the edges) — so cores run fully independently, no
collectives. Per 128-query block, attention is computed against a 640-row
key window instead of all 6144 keys (12x less tensor work than dense).

On-device math (per core): LN1 -> q/k/v projections (bf16 matmuls, fp32
accum) -> band-masked augmented-logit attention (q.k/sqrt(D) + cos-sim via
concatenated [q | hn] contraction) -> fp32 softmax with isolated-node
fallback to v -> residual + Wo -> LN2 -> FFN(gelu) -> residual. The grid
band mask (Chebyshev distance <= 2, self excluded) is computed on device
from the grid coordinates; the self-exclusion diagonal enters the logit
accumulation as a constant matmul.
"""
import os
import sys

import numpy as np

sys.path.insert(0, "/opt/trn_rl_repo")

import ml_dtypes  # noqa: E402
from contextlib import ExitStack  # noqa: E402

import concourse.bacc as bacc  # noqa: E402
import concourse.bass as bass  # noqa: E402
import concourse.tile as tile  # noqa: E402
from concourse import mybir  # noqa: E402
from concourse.masks import make_identity  # noqa: E402
from concourse.bass_utils import run_bass_kernel_spmd  # noqa: E402

P = 128
D = 512
DC = D // P            # 4 d-chunks
DFF = 2048
FC = DFF // P          # 16 ffn chunks
N = 6144
N_CORES = 8
ROWS = N // N_CORES    # 768 query rows per core
NB = ROWS // P         # 6 query blocks per core
H = 256                # halo rows each side (multiple of 128)
KR = ROWS + 2 * H      # 1280 key rows per core
NT = KR // P           # 10 key chunks
W = H + P + H          # 640 key window per query block
GR = 384               # ffn row group (fits one PSUM bank)
PEN = 30000.0
LN_EPS = 1e-5
SIM_BETA = 1.0
RADIUS = 2
F32 = mybir.dt.float32
BF16 = mybir.dt.bfloat16
AX = mybir.AxisListType
OP = mybir.AluOpType
ACT = mybir.ActivationFunctionType

LAST_EXEC_NS = None


def build_program(affine1: bool, affine2: bool):
    """affine1/affine2: apply ln1/ln2 gamma-beta on device (False when the
    inputs are exactly ones/zeros, which makes them identities)."""
    nc = bacc.Bacc("TRN2", target_bir_lowering=False, debug=False,
                   num_devices=N_CORES)
    dt_in = lambda name, shape, dt: nc.dram_tensor(name, shape, dt,
                                                   kind="ExternalInput").ap()
    xh = dt_in("xh", [KR, D], F32)
    gq = dt_in("gq", [ROWS, 2], F32)
    gxkb = dt_in("gxkb", [P, KR], BF16)
    gykb = dt_in("gykb", [P, KR], BF16)
    wq = dt_in("wq", [P, DC, D], BF16)   # pre-scaled by 1/sqrt(D) on host
    wk = dt_in("wk", [P, DC, D], BF16)
    wv = dt_in("wv", [P, DC, D], BF16)
    wo = dt_in("wo", [P, DC, D], BF16)
    w1 = dt_in("w1", [P, DC, DFF], BF16)
    w2 = dt_in("w2", [P, FC, D], BF16)
    bqs = dt_in("bqs", [P, DC], F32)     # (bq / sqrt(D)) chunked [p, dc]
    bks = dt_in("bks", [P, DC], F32)
    b1s = dt_in("b1s", [P, FC], F32)
    bv = dt_in("bv", [P, D], BF16)
    bo16 = dt_in("bo16", [1, D], BF16)
    b2 = dt_in("b2", [P, D], F32)
    if affine1:
        l1g = dt_in("l1g", [P, D], BF16)
        l1b = dt_in("l1b", [P, D], BF16)
    if affine2:
        l2g = dt_in("l2g", [P, D], BF16)
        l2b = dt_in("l2b", [P, D], BF16)
    out = nc.dram_tensor("out", [ROWS, D], F32, kind="ExternalOutput").ap()

    with tile.TileContext(nc) as tc, ExitStack() as ctx:
        const = ctx.enter_context(tc.tile_pool(name="const", bufs=1))
        big = ctx.enter_context(tc.tile_pool(name="big", bufs=1))
        temps = ctx.enter_context(tc.tile_pool(name="temps", bufs=3))
        small = ctx.enter_context(tc.tile_pool(name="small", bufs=6))
        psA = ctx.enter_context(tc.tile_pool(name="psA", bufs=4, space="PSUM"))
        psS = ctx.enter_context(tc.tile_pool(name="psS", bufs=2, space="PSUM"))

        # float biases used by scalar.activation need registered const APs
        for val in (0.0, LN_EPS, 1e-16):
            ct = const.tile([P, 1], F32, tag=f"const_{val}")
            nc.vector.memset(ct, val)
            nc.const_aps.aps[(F32, val)] = ct

        # ---- small constants first (scalar queue), then early weights
        bqs_t = const.tile([P, DC], F32)
        bks_t = const.tile([P, DC], F32)
        b1s_t = const.tile([P, FC], F32)
        gq_t = const.tile([P, NB, 2], F32)
        gxk_bc = const.tile([P, KR], BF16)
        gyk_bc = const.tile([P, KR], BF16)
        bv_bc = const.tile([P, D], BF16)
        b2_bc = const.tile([P, D], F32)
        bo16_t = const.tile([1, D], BF16)
        loads = [(bqs_t, bqs), (bks_t, bks), (b1s_t, b1s),
                 (gxk_bc, gxkb), (gyk_bc, gykb), (bv_bc, bv),
                 (b2_bc, b2), (bo16_t, bo16)]
        if affine1:
            l1g_bc = const.tile([P, D], BF16)
            l1b_bc = const.tile([P, D], BF16)
            loads += [(l1g_bc, l1g), (l1b_bc, l1b)]
        if affine2:
            l2g_bc = const.tile([P, D], BF16)
            l2b_bc = const.tile([P, D], BF16)
            loads += [(l2g_bc, l2g), (l2b_bc, l2b)]
        for t_, a_ in loads:
            nc.scalar.dma_start(out=t_, in_=a_)
        nc.scalar.dma_start(out=gq_t, in_=gq.rearrange("(nb p) c -> p nb c", p=P))

        wq_t = const.tile([P, DC, D], BF16)
        wk_t = const.tile([P, DC, D], BF16)
        wv_t = const.tile([P, DC, D], BF16)
        wo_t = const.tile([P, DC, D], BF16)
        w1_t = const.tile([P, DC, DFF], BF16)
        w2_t = const.tile([P, FC, D], BF16)
        for t_, a_ in [(wv_t, wv), (wk_t, wk), (wq_t, wq)]:
            nc.scalar.dma_start(out=t_, in_=a_)

        # identity (for the constant diag matmul) + shifted self-penalty row
        ident = const.tile([P, P], BF16)
        make_identity(nc, ident)
        shiftpen = const.tile([P, W], BF16)
        nc.gpsimd.memset(shiftpen, 0.0)
        nc.gpsimd.affine_select(out=shiftpen, in_=shiftpen,
                                compare_op=OP.not_equal, fill=-PEN,
                                base=H, channel_multiplier=1,
                                pattern=[[-1, W]])
        ones1 = const.tile([1, P], BF16)
        nc.vector.memset(ones1, 1.0)

        # ---- persistent activations
        HT = big.tile([P, DC, KR], BF16, tag="slotA")
        HnT = big.tile([P, DC, KR], BF16, tag="slotB")
        qt = big.tile([P, DC, ROWS], BF16)
        kt = big.tile([P, DC, KR], BF16)
        vt = big.tile([P, NT, D], BF16)
        aoT = big.tile([P, DC, ROWS], BF16)
        h2T = big.tile([P, DC, ROWS], BF16)
        gt = big.tile([P, FC, ROWS], BF16, tag="slotA")  # reuses HT slot
        x2t = big.tile([P, NB, D], F32, tag="slotB")     # reuses HnT slot

        def kq_rowblock(kind, r0, rn_):
            """Emit one q^T/k^T row-block: 4 dout-chunks x 4 accum matmuls."""
            src_t, dst, bias, off = ((wk_t, kt, bks_t, 0) if kind == "k"
                                     else (wq_t, qt, bqs_t, H))
            for dcx in range(DC):
                ps = psA.tile([P, 512], F32, tag="ps")
                for ci in range(DC):
                    nc.tensor.matmul(ps[:, :rn_],
                                     src_t[:, ci, dcx * P:(dcx + 1) * P],
                                     HT[:, ci, off + r0:off + r0 + rn_],
                                     start=(ci == 0), stop=(ci == DC - 1))
                nc.scalar.activation(dst[:, dcx, r0:r0 + rn_], ps[:, :rn_],
                                     ACT.Identity, bias=bias[:, dcx:dcx + 1],
                                     scale=1.0)

        # after finishing LN chunk t, these k/q row-blocks have their inputs
        kq_sched = {3: [("k", 0, 512)], 5: [("q", 0, 512)],
                    7: [("k", 512, 512), ("q", 512, 256)],
                    9: [("k", 1024, 256)]}

        # ---- Phase A+B interleaved: LN1, transposes, projections
        for t in range(NT):
            xt = temps.tile([P, D], F32, tag="xt")
            nc.sync.dma_start(out=xt, in_=xh[t * P:(t + 1) * P, :])
            st = small.tile([P, 6], F32, tag="st")
            nc.vector.bn_stats(st, xt)
            mv = small.tile([P, 2], F32, tag="mv")
            nc.vector.bn_aggr(mv, st)
            sd = small.tile([P, 1], F32, tag="sd")
            nc.scalar.activation(sd, mv[:, 1:2], ACT.Sqrt, bias=LN_EPS)
            rstd = small.tile([P, 1], F32, tag="rstd")
            nc.vector.reciprocal(rstd, sd)
            hb = temps.tile([P, D], BF16, tag="hb")
            nc.vector.tensor_scalar(hb, xt, mv[:, 0:1], rstd,
                                    op0=OP.subtract, op1=OP.mult)
            if affine1:
                nc.vector.tensor_tensor(hb, hb, l1g_bc, op=OP.mult)
                nc.vector.tensor_tensor(hb, hb, l1b_bc, op=OP.add)
            scr = temps.tile([P, D], BF16, tag="scr")
            h2s = small.tile([P, 1], F32, tag="h2s")
            nc.scalar.activation(scr, hb, ACT.Square, accum_out=h2s)
            nsq = small.tile([P, 1], F32, tag="nsq")
            nc.scalar.activation(nsq, h2s, ACT.Sqrt, bias=1e-16)
            rn = small.tile([P, 1], F32, tag="rn")
            nc.vector.reciprocal(rn, nsq)
            hnb = temps.tile([P, D], BF16, tag="hnb")
            nc.vector.tensor_scalar_mul(hnb, hb, rn)
            nc.sync.dma_start_transpose(HT[:, :, t * P:(t + 1) * P], hb)
            nc.scalar.dma_start_transpose(HnT[:, :, t * P:(t + 1) * P], hnb)
            # v rows for this chunk
            ps = psA.tile([P, 512], F32, tag="ps")
            for ci in range(DC):
                nc.tensor.matmul(ps, HT[:, ci, t * P:(t + 1) * P],
                                 wv_t[:, ci, :],
                                 start=(ci == 0), stop=(ci == DC - 1))
            nc.vector.tensor_tensor(vt[:, t, :], ps, bv_bc, op=OP.add)
            for kind, r0, rn_ in kq_sched.get(t, []):
                kq_rowblock(kind, r0, rn_)

        # ---- Phase C: band attention per 128-query block
        for b in range(NB):
            wb = P * b  # local window start (own queries sit at H + wb)
            S = psS.tile([P, W], F32, tag="S")
            for c0, cn in [(0, 512), (512, 128)]:
                # constant self-exclusion diagonal enters the accumulation
                nc.tensor.matmul(S[:, c0:c0 + cn], ident,
                                 shiftpen[:, c0:c0 + cn], start=True,
                                 stop=False)
                for ac in range(2 * DC):
                    if ac < DC:
                        lhsT = qt[:, ac, wb:wb + P]
                        rhs = kt[:, ac, wb + c0:wb + c0 + cn]
                    else:
                        lhsT = HnT[:, ac - DC, H + wb:H + wb + P]
                        rhs = HnT[:, ac - DC, wb + c0:wb + c0 + cn]
                    nc.tensor.matmul(S[:, c0:c0 + cn], lhsT, rhs,
                                     start=False, stop=(ac == 2 * DC - 1))
            # band mask from grid coords (gpsimd+vector split)
            dgx = temps.tile([P, W], BF16, tag="dgx")
            nc.vector.tensor_scalar(dgx, gxk_bc[:, wb:wb + W],
                                    gq_t[:, b, 0:1], None, op0=OP.subtract)
            dgy = temps.tile([P, W], BF16, tag="dgy")
            nc.vector.tensor_scalar(dgy, gyk_bc[:, wb:wb + W],
                                    gq_t[:, b, 1:2], None, op0=OP.subtract)
            mn = temps.tile([P, W], BF16, tag="mn")
            nc.vector.tensor_tensor(mn, dgx, dgy, op=OP.min)
            nc.vector.tensor_tensor(dgx, dgx, dgy, op=OP.max)
            nc.vector.tensor_scalar(dgy, dgx, RADIUS + 0.5, None, op0=OP.is_le)
            vcnt = small.tile([P, 1], F32, tag="vcnt")
            nc.vector.scalar_tensor_tensor(mn, mn, -(RADIUS + 0.5), dgy,
                                           op0=OP.is_ge, op1=OP.mult,
                                           accum_out=vcnt)
            nc.vector.scalar_tensor_tensor(S, mn, PEN, S,
                                           op0=OP.mult, op1=OP.add)
            m_ = small.tile([P, 1], F32, tag="m_")
            nc.vector.tensor_reduce(m_, S, axis=AX.X, op=OP.max)
            negm = small.tile([P, 1], F32, tag="negm")
            nc.vector.tensor_scalar(negm, m_, -1.0, None, op0=OP.mult)
            pb = temps.tile([P, W], BF16, tag="pb")
            srow = small.tile([P, 1], F32, tag="srow")
            nc.scalar.activation(pb, S, ACT.Exp, bias=negm, scale=1.0,
                                 accum_out=srow)
            rs = small.tile([P, 1], F32, tag="rs")
            nc.vector.reciprocal(rs, srow)
            PT = temps.tile([P, W // P, P], BF16, tag="PT")
            nc.sync.dma_start_transpose(PT[:, :, :], pb)
            po = psA.tile([P, 512], F32, tag="ps")
            for j in range(W // P):
                nc.tensor.matmul(po, PT[:, j, :], vt[:, b + j, :],
                                 start=(j == 0), stop=(j == W // P - 1))
            has = small.tile([P, 1], F32, tag="has")
            nc.vector.tensor_scalar(has, vcnt, 1.5, None, op0=OP.is_ge)
            o1 = temps.tile([P, D], F32, tag="o1")
            nc.vector.scalar_tensor_tensor(o1, po, rs, vt[:, H // P + b, :],
                                           op0=OP.mult, op1=OP.subtract)
            aob = temps.tile([P, D], BF16, tag="aob")
            nc.vector.scalar_tensor_tensor(aob, o1, has, vt[:, H // P + b, :],
                                           op0=OP.mult, op1=OP.add)
            nc.scalar.dma_start_transpose(aoT[:, :, b * P:(b + 1) * P], aob)

        # late weights (needed from phase D/E onward)
        nc.scalar.dma_start(out=wo_t, in_=wo)
        nc.scalar.dma_start(out=w1_t, in_=w1)
        nc.sync.dma_start(out=w2_t, in_=w2)

        def phase_d_block(b):
            ps = psA.tile([P, 512], F32, tag="ps")
            nc.tensor.matmul(ps, ones1, bo16_t, start=True, stop=False)  # + bo
            for ci in range(DC):
                nc.tensor.matmul(ps, aoT[:, ci, b * P:(b + 1) * P],
                                 wo_t[:, ci, :],
                                 start=False, stop=(ci == DC - 1))
            xt = temps.tile([P, D], F32, tag="xt2")
            nc.gpsimd.dma_start(out=xt, in_=xh[H + b * P:H + (b + 1) * P, :])
            x2 = x2t[:, b, :]
            nc.vector.scalar_tensor_tensor(x2, ps, 0.0, xt,
                                           op0=OP.bypass, op1=OP.add)
            st = small.tile([P, 6], F32, tag="st")
            nc.vector.bn_stats(st, x2)
            mv = small.tile([P, 2], F32, tag="mv")
            nc.vector.bn_aggr(mv, st)
            sd = small.tile([P, 1], F32, tag="sd")
            nc.scalar.activation(sd, mv[:, 1:2], ACT.Sqrt, bias=LN_EPS)
            rstd = small.tile([P, 1], F32, tag="rstd")
            nc.vector.reciprocal(rstd, sd)
            h2b = temps.tile([P, D], BF16, tag="hb")
            nc.vector.tensor_scalar(h2b, x2, mv[:, 0:1], rstd,
                                    op0=OP.subtract, op1=OP.mult)
            if affine2:
                nc.vector.tensor_tensor(h2b, h2b, l2g_bc, op=OP.mult)
                nc.vector.tensor_tensor(h2b, h2b, l2b_bc, op=OP.add)
            nc.sync.dma_start_transpose(h2T[:, :, b * P:(b + 1) * P], h2b)

        def ffn_in_group(g0):
            for fcx in range(FC):
                ps = psA.tile([P, 512], F32, tag="ps")
                for ci in range(DC):
                    nc.tensor.matmul(ps[:, :GR],
                                     w1_t[:, ci, fcx * P:(fcx + 1) * P],
                                     h2T[:, ci, g0:g0 + GR],
                                     start=(ci == 0), stop=(ci == DC - 1))
                nc.scalar.activation(gt[:, fcx, g0:g0 + GR], ps[:, :GR],
                                     ACT.Gelu, bias=b1s_t[:, fcx:fcx + 1],
                                     scale=1.0)

        def ffn_out_block(b):
            ps = psA.tile([P, 512], F32, tag="ps")
            for fcx in range(FC):
                nc.tensor.matmul(ps, gt[:, fcx, b * P:(b + 1) * P],
                                 w2_t[:, fcx, :],
                                 start=(fcx == 0), stop=(fcx == FC - 1))
            fo = temps.tile([P, D], F32, tag="fo")
            nc.vector.scalar_tensor_tensor(fo, ps, 0.0, x2t[:, b, :],
                                           op0=OP.bypass, op1=OP.add)
            nc.vector.tensor_tensor(fo, fo, b2_bc, op=OP.add)
            nc.sync.dma_start(out=out[b * P:(b + 1) * P, :], in_=fo)

        # ---- Phases D+E interleaved to keep the PE dense
        for b in (0, 1, 2):
            phase_d_block(b)
        ffn_in_group(0)
        for b in (3, 4, 5):
            phase_d_block(b)
        for b in (0, 1, 2):
            ffn_out_block(b)
        ffn_in_group(GR)
        for b in (3, 4, 5):
            ffn_out_block(b)

    nc.compile()
    return nc


_prog = {}


def _get_program(affine1, affine2):
    key = (affine1, affine2)
    if key not in _prog:
        _prog[key] = build_program(affine1, affine2)
    return _prog[key]


def _np_fallback(x, grid, Wq, bq, Wk, bk, Wv, bv, Wo, bo,
                 ln1_g, ln1_b, ln2_g, ln2_b, W1, b1, W2, b2):
    """Exact fp64 host path (only used if an input violates the band bound)."""
    from scipy.special import erf
    x = np.asarray(x, np.float64)
    g = np.asarray(grid).astype(np.float64)

    def ln(v, gm, bt, eps=1e-5):
        mu = v.mean(-1, keepdims=True)
        var = v.var(-1, keepdims=True)
        return (v - mu) / np.sqrt(var + eps) * gm + bt

    h = ln(x, ln1_g, ln1_b)
    q = h @ Wq + bq
    k = h @ Wk + bk
    v = h @ Wv + bv
    hn = h / np.maximum(np.linalg.norm(h, axis=-1, keepdims=True), 1e-8)
    scale = 1.0 / np.sqrt(D)
    n = x.shape[0]
    outp = np.empty_like(x)
    for s in range(0, n, 512):
        e = min(s + 512, n)
        dx = np.abs(g[s:e, None, 0] - g[None, :, 0])
        dy = np.abs(g[s:e, None, 1] - g[None, :, 1])
        mask = (dx <= RADIUS) & (dy <= RADIUS)
        mask[np.arange(e - s), np.arange(s, e)] = False
        logits = (q[s:e] @ k.T) * scale + SIM_BETA * (hn[s:e] @ hn.T)
        logits = np.where(mask, logits, -1e30)
        m = logits.max(-1, keepdims=True)
        p = np.exp(logits - m)
        att = p / p.sum(-1, keepdims=True)
        o = att @ v
        outp[s:e] = np.where(mask.any(1, keepdims=True), o, v[s:e])
    x = x + outp @ Wo + bo
    h2 = ln(x, ln2_g, ln2_b)
    a = h2 @ W1 + b1
    gelu = 0.5 * a * (1.0 + erf(a / np.sqrt(2.0)))
    return (x + gelu @ W2 + b2).astype(np.float32)


def kernel(x, grid, Wq, bq, Wk, bk, Wv, bv, Wo, bo,
           ln1_g, ln1_b, ln2_g, ln2_b, W1, b1, W2, b2):
    global LAST_EXEC_NS
    x = np.ascontiguousarray(np.asarray(x, np.float32))
    g = np.asarray(grid).astype(np.int64)

    # ---- host-side sharding: sort rows by (gx, gy)
    perm = np.lexsort((g[:, 1], g[:, 0]))
    inv_perm = np.argsort(perm)
    gs = g[perm]
    xs = x[perm]

    # band-coverage check: all neighbours of block b within [128b-H, 128b+128+H)
    ok = True
    gx = gs[:, 0]
    for b in range(N // P):
        lo = np.searchsorted(gx, gx[b * P:(b + 1) * P].min() - RADIUS, "left")
        hi = np.searchsorted(gx, gx[b * P:(b + 1) * P].max() + RADIUS, "right")
        if b * P - lo > H or hi - (b * P + P) > H:
            ok = False
            break
    if not ok:
        return _np_fallback(x, grid, Wq, bq, Wk, bk, Wv, bv, Wo, bo,
                            ln1_g, ln1_b, ln2_g, ln2_b, W1, b1, W2, b2)

    lam = 1.0 / np.sqrt(D)
    affine1 = not (np.all(np.asarray(ln1_g) == 1.0)
                   and np.all(np.asarray(ln1_b) == 0.0))
    affine2 = not (np.all(np.asarray(ln2_g) == 1.0)
                   and np.all(np.asarray(ln2_b) == 0.0))

    b16 = lambda a: np.ascontiguousarray(np.asarray(a)).astype(ml_dtypes.bfloat16)
    f32 = lambda a: np.ascontiguousarray(np.asarray(a, np.float32))
    bcast16 = lambda a: np.ascontiguousarray(
        np.broadcast_to(np.asarray(a).astype(ml_dtypes.bfloat16), (P, D)))

    # weights in [p, chunk, out] layout (p = contraction index % 128)
    chunked = lambda w_, nch: np.ascontiguousarray(
        b16(w_).reshape(nch, P, -1).transpose(1, 0, 2))
    wq_h = chunked(np.asarray(Wq, np.float64) * lam, DC)
    wk_h, wv_h, wo_h = (chunked(w_, DC) for w_ in (Wk, Wv, Wo))
    w1_h = chunked(W1, DC)
    w2_h = chunked(W2, FC)
    bqs_h = np.ascontiguousarray((np.asarray(bq, np.float64) * lam)
                                 .astype(np.float32).reshape(DC, P).T)
    bks_h = np.ascontiguousarray(f32(bk).reshape(DC, P).T)
    b1s_h = np.ascontiguousarray(f32(b1).reshape(FC, P).T)

    shared = dict(wq=wq_h, wk=wk_h, wv=wv_h, wo=wo_h, w1=w1_h, w2=w2_h,
                  bqs=bqs_h, bks=bks_h, b1s=b1s_h,
                  bv=bcast16(bv), bo16=b16(bo).reshape(1, D),
                  b2=np.ascontiguousarray(
                      np.broadcast_to(f32(b2), (P, D))))
    if affine1:
        shared.update(l1g=bcast16(ln1_g), l1b=bcast16(ln1_b))
    if affine2:
        shared.update(l2g=bcast16(ln2_g), l2b=bcast16(ln2_b))

    in_maps = []
    gs_f = gs.astype(np.float32)
    for c in range(N_CORES):
        glo = c * ROWS - H
        xh_c = np.zeros((KR, D), np.float32)
        gk_c = np.full((KR, 2), 10000.0, np.float32)
        s0, s1 = max(0, glo), min(N, glo + KR)
        xh_c[s0 - glo:s1 - glo] = xs[s0:s1]
        gk_c[s0 - glo:s1 - glo] = gs_f[s0:s1]
        gkb = gk_c.astype(ml_dtypes.bfloat16)
        in_maps.append(dict(shared,
                            xh=xh_c,
                            gxkb=np.ascontiguousarray(
                                np.broadcast_to(gkb[:, 0], (P, KR))),
                            gykb=np.ascontiguousarray(
                                np.broadcast_to(gkb[:, 1], (P, KR))),
                            gq=gs_f[c * ROWS:(c + 1) * ROWS]))

    nc = _get_program(affine1, affine2)
    tmpdir = os.environ.get("KERNEL_TRACE_DIR") or None
    res = run_bass_kernel_spmd(nc, in_maps, list(range(N_CORES)),
                               tmpdir=tmpdir)
    LAST_EXEC_NS = res.exec_time_ns
    out_sorted = np.concatenate([res.results[c]["out"]
                                 for c in range(N_CORES)], axis=0)
    return np.ascontiguousarray(out_sorted[inv_perm]).astype(np.float32)
